# revision 2
# baseline (speedup 1.0000x reference)
"""DPConv (kernel=8, ext=4, stride=4) on 8 TRN2 NeuronCores.

Math: with K = k + 2e = 16 and k = 8, every adaptive-pool bin is exactly
2 wide, so the whole DPConv collapses to a separable linear operator:

    out_img = L @ img @ L.T          (per (n, c) image, 128x128)

where L is a 128x128 stencil matrix: for output index w the contributing
windows are i in [max(0,ceil((w-7)/4)), min(30, floor(w/4))] (counted
twice when that range is a single i - the fold count normalization),
each contributing the clamped replicate-padded pair {2w-4i-4, 2w-4i-3}
with weight 1/4 (pool avg 1/2 x fold avg 1/2).

This problem is memory-bound, so the kernel trades precision it does not
need (gate: rel err < 2e-2; this pipeline measures ~4e-3) for HBM bytes:
the input is cast to bf16 on the host and the output is written bf16 and
upcast on the host, halving both directions vs fp32 (2 MiB in + 2 MiB
out per core).

On-chip per image tile [H=128 partitions, W=128 free]:
  rows:  T = (L/4) @ x on TensorE, one bf16 matmul per 8-image group
         (PSUM fp32 accumulate; L is exact in bf16 - entries are n/16).
  cols:  P[v] = T[2v-4]+T[2v-3]  (pairsum: ACT evacuates even cols
         PSUM->SBUF, DVE adds odd cols straight from PSUM)
         out[4a+b] = P[2a+b] + P[2a+b+2]  (fold: overlapping as-strided
         reads, contiguous bf16 write; split between DVE and GpSimd so
         neither paces the chain)
  edge columns use x2 scaled copies on ACT / broadcast reads.

Sharding: pure data parallel - core k takes batch element n = k.
Host staging per core: transpose to [H, C, W] (so DMA reads are 2 KiB
contiguous per partition) and cast to bf16; output arrives [H, C, W]
bf16 and is upcast + transposed back on the host.
"""

import ml_dtypes
import numpy as np

import concourse.bacc as bacc
import concourse.mybir as mybir
import concourse.tile as tile
from concourse import bass_utils
from concourse.ap import AP

N_CORES = 8
C_PER_CORE = 64          # images per core (= C; one batch element per core)
G = 8                    # images per compute group
N_GROUPS = C_PER_CORE // G
F32 = mybir.dt.float32
BF16 = mybir.dt.bfloat16
BF16_NP = ml_dtypes.bfloat16


def _build_lq() -> np.ndarray:
    """The 1-D DPConv operator with both 1/4 scalings folded in: L/4."""
    L = np.zeros((128, 128), np.float64)
    for w in range(128):
        i_lo = max(0, -((7 - w) // 4))      # ceil((w-7)/4)
        i_hi = min(30, w // 4)
        for i in (i_lo, i_hi):              # counted twice when equal
            L[w, min(127, max(0, 2 * w - 4 * i - 4))] += 0.25
            L[w, min(127, max(0, 2 * w - 4 * i - 3))] += 0.25
    return (L / 4.0).astype(np.float32)


_LQ_T = np.ascontiguousarray(_build_lq().T)          # lhsT layout [r, h]
_LQ_T_BF16 = _LQ_T.astype(BF16_NP)
assert np.all(_LQ_T_BF16.astype(np.float32) == _LQ_T)  # L exact in bf16


def _as_strided(base: AP, dims) -> AP:
    """Rebuild `base` (a sliced AP pointing at the wanted offset) with
    explicit [stride, size] free dims (overlapping reads allowed)."""
    return AP(base.tensor, base.offset, dims)


def _dpconv_tile(tc, o_d, x_d, lt_d):
    nc = tc.nc
    with tc.tile_pool(name="const", bufs=1) as cp, \
         tc.tile_pool(name="in", bufs=8) as inp, \
         tc.tile_pool(name="io", bufs=4) as iop, \
         tc.tile_pool(name="mid", bufs=4) as mp, \
         tc.tile_pool(name="ps", bufs=3, space="PSUM") as pp:
        lt = cp.tile([128, 128], BF16)
        nc.sync.dma_start(out=lt[:], in_=lt_d)
        for g in range(N_GROUPS):
            # input arrives in 8-image 256 KiB chunks on the sync/SP
            # HWDGE ring (FIFO): all 8 issue up-front (bufs=8) so the
            # ring streams back-to-back while compute drains it.
            sl = slice(g * G, (g + 1) * G)
            ct = inp.tile([128, G, 128], BF16, tag="in")
            nc.sync.dma_start(out=ct[:], in_=x_d[:, sl, :])

            # rows: T = (L/4) @ x, one 512-col matmul per 4-image half
            t1 = pp.tile([128, G, 128], F32, tag="t1")
            nc.tensor.matmul(t1[:, 0:4, :], lt[:], ct[:, 0:4, :],
                             start=True, stop=True)
            nc.tensor.matmul(t1[:, 4:8, :], lt[:], ct[:, 4:8, :],
                             start=True, stop=True)

            # cols step 1: pairsum P[v] = T[2v-4] + T[2v-3] (clamped).
            # TensorTensor may read at most ONE input from PSUM: ACT
            # (close to PSUM, otherwise idle) evacuates the even cols,
            # DVE adds the odd cols from PSUM. Edge cols are x2 scaled
            # broadcast copies on ACT.
            pe_t = mp.tile([128, G, 64], F32, tag="pe")
            nc.scalar.copy(out=pe_t[:], in_=t1[:, :, 0:128:2])
            pt = mp.tile([128, G, 68], F32, tag="P")
            gdim = list(pt[:].ap[1])            # [68-ish pitch, G]
            pdim0 = list(pt[:].ap[0])           # partition dim
            tdim = t1[:].ap
            nc.vector.tensor_add(
                out=pt[:, :, 2:66], in0=pe_t[:], in1=t1[:, :, 1:128:2])
            # P edge cols {0,1,66,67} = 2x T cols {0,0,127,127}: one ACT
            # op - out strides (66,1), in strides (127, 0-broadcast)
            nc.scalar.mul(
                _as_strided(pt[:, :, 0:1], [pdim0, gdim, [66, 2], [1, 2]]),
                _as_strided(t1[:, :, 0:1],
                            [list(tdim[0]), list(tdim[1]), [127, 2], [0, 2]]),
                2.0)

            # cols step 2: fold out[4a+b] = P[2a+b] + P[2a+b+2], with
            # overlapping as-strided reads (a x2, b x1) and a contiguous
            # bf16 write of cols 4..123 - split between DVE (a=0..14)
            # and GpSimd (a=15..29) so neither paces the chain. Edge
            # cols {0..3,124..127} = 2x P{0..3,64..67} ride ACT as a
            # scaled two-region copy.
            ot = iop.tile([128, G, 128], BF16, tag="out")
            odim = ot[:].ap
            in0a = _as_strided(pt[:, :, 2:3], [pdim0, gdim, [2, 15], [1, 4]])
            in1a = _as_strided(pt[:, :, 4:5], [pdim0, gdim, [2, 15], [1, 4]])
            outa = _as_strided(
                ot[:, :, 4:5], [list(odim[0]), list(odim[1]), [4, 15], [1, 4]])
            nc.vector.tensor_add(out=outa, in0=in0a, in1=in1a)
            in0b = _as_strided(pt[:, :, 32:33], [pdim0, gdim, [2, 15], [1, 4]])
            in1b = _as_strided(pt[:, :, 34:35], [pdim0, gdim, [2, 15], [1, 4]])
            outb = _as_strided(
                ot[:, :, 64:65], [list(odim[0]), list(odim[1]), [4, 15], [1, 4]])
            nc.gpsimd.tensor_add(out=outb, in0=in0b, in1=in1b)
            edge_in = _as_strided(pt[:, :, 0:1], [pdim0, gdim, [64, 2], [1, 4]])
            edge_out = _as_strided(
                ot[:, :, 0:1], [list(odim[0]), list(odim[1]), [124, 2], [1, 4]])
            nc.scalar.mul(edge_out, edge_in, 2.0)

            # stores ride the ACT HWDGE ring so they never FIFO behind
            # upcoming loads on the SP ring; the output stays H-major
            # ([H,C,W], un-transposed on the host) so each partition
            # writes one 2 KiB contiguous run
            nc.scalar.dma_start(out=o_d[:, sl, :], in_=ot[:])


_CACHE = {}


def _get_nc():
    if "nc" not in _CACHE:
        nc = bacc.Bacc("TRN2", target_bir_lowering=False, debug=False)
        x_d = nc.dram_tensor("x", (128, C_PER_CORE, 128), BF16,
                             kind="ExternalInput").ap()
        lt_d = nc.dram_tensor("lt", (128, 128), BF16,
                              kind="ExternalInput").ap()
        o_d = nc.dram_tensor("o", (128, C_PER_CORE, 128), BF16,
                             kind="ExternalOutput").ap()
        with tile.TileContext(nc) as tc:
            _dpconv_tile(tc, o_d, x_d, lt_d)
        nc.compile()
        _CACHE["nc"] = nc
    return _CACHE["nc"]


def _stage(xk: np.ndarray) -> np.ndarray:
    """[C,H,W] f32 -> [H,C,W] bf16 (H-major so DMA lines are contiguous)."""
    return np.ascontiguousarray(xk.transpose(1, 0, 2)).astype(BF16_NP)


def run(x: np.ndarray, **spmd_kwargs) -> bass_utils.BassKernelResults:
    """Shard x (8,64,128,128) across 8 cores and run the Bass kernel."""
    nc = _get_nc()
    in_maps = [
        {"x": _stage(x[k]), "lt": _LQ_T_BF16} for k in range(N_CORES)
    ]
    return bass_utils.run_bass_kernel_spmd(
        nc, in_maps, core_ids=list(range(N_CORES)), **spmd_kwargs)


def kernel(x) -> np.ndarray:
    x = np.asarray(x, dtype=np.float32)
    assert x.shape == (N_CORES, C_PER_CORE, 128, 128), x.shape
    res = run(x)
    return np.stack(
        [res.results[k]["o"].astype(np.float32).transpose(1, 0, 2)
         for k in range(N_CORES)],
        axis=0)


# revision 3
# speedup vs baseline: 1.0477x; 1.0477x over previous
"""DPConv (kernel=8, ext=4, stride=4) on 8 TRN2 NeuronCores.

Math: with K = k + 2e = 16 and k = 8, every adaptive-pool bin is exactly
2 wide, so the whole DPConv collapses to a separable linear operator:

    out_img = L @ img @ L.T          (per (n, c) image, 128x128)

The H-side L is a matmul (contraction over partitions). The W-side L
factors as fold(F) . pairsum(S); both act on the free axis and commute
with the H-side matmul, so S is folded INTO the matmul: PSUM accumulates
P = LQ @ x_even + LQ @ x_odd (two matmuls, strided rhs reads) and only
F (the fold) plus edge scaling remain as vector work.

Memory regime: the gate is rel err < 2e-2 and this pipeline measures
~5e-3, so I/O is bf16 both ways (host casts in, host upcasts out),
halving HBM traffic vs fp32 to 2 MiB in + 2 MiB out per core.

Per 16-image group [128 partitions = H, free = (c, W)]:
  mm   P_main[c,64] = LQ @ x[:,even] (+)= LQ @ x[:,odd]   4x 512-free
       P_edge[c,2]  = LQ @ x[:,{0,127}]                   1x 32-free
  evac psb[c,2:66] <- P_main (ACT half / DVE half, bf16)
       psb[c,{0,1,66,67}] <- 2*P_edge (ACT, broadcast strided)
  fold out[4a+b] = P[2a+b] + P[2a+b+2]: overlapping strided reads,
       contiguous bf16 writes, split DVE (a<16) / GpSimd (a>=16)
       out[{0..3,124..127}] = 2*P[{0..3,64..67}] on ACT
  DMA  in on the sync/SP HWDGE ring, out on the ACT ring (no FIFO
       interference), 512 KiB per transfer.

Sharding: pure data parallel - core k takes batch element n = k.
Host staging per core: transpose to [H, C, W] (2 KiB contiguous DMA
lines) + bf16 cast; output returns [H, C, W] bf16, host upcasts.
"""

import ml_dtypes
import numpy as np

import concourse.bacc as bacc
import concourse.mybir as mybir
import concourse.tile as tile
from concourse import bass_utils
from concourse.ap import AP

N_CORES = 8
C_PER_CORE = 64          # images per core (= C; one batch element per core)
G = 16                   # images per compute group
N_GROUPS = C_PER_CORE // G
DVE_A = 16               # fold rows 0..DVE_A-1 on DVE, rest on GpSimd
F32 = mybir.dt.float32
BF16 = mybir.dt.bfloat16
BF16_NP = ml_dtypes.bfloat16


def _build_lq() -> np.ndarray:
    """The 1-D DPConv operator with both 1/4 scalings folded in: L/4."""
    L = np.zeros((128, 128), np.float64)
    for w in range(128):
        i_lo = max(0, -((7 - w) // 4))      # ceil((w-7)/4)
        i_hi = min(30, w // 4)
        for i in (i_lo, i_hi):              # counted twice when equal
            L[w, min(127, max(0, 2 * w - 4 * i - 4))] += 0.25
            L[w, min(127, max(0, 2 * w - 4 * i - 3))] += 0.25
    return (L / 4.0).astype(np.float32)


_LQ_T = np.ascontiguousarray(_build_lq().T)          # lhsT layout [r, h]
_LQ_T_BF16 = _LQ_T.astype(BF16_NP)
assert np.all(_LQ_T_BF16.astype(np.float32) == _LQ_T)  # L exact in bf16


def _as_strided(base: AP, dims) -> AP:
    """Rebuild `base` (a sliced AP pointing at the wanted offset) with
    explicit [stride, size] free dims (overlapping reads allowed)."""
    return AP(base.tensor, base.offset, dims)


def _dpconv_tile(tc, o_d, x_d, lt_d):
    nc = tc.nc
    with tc.tile_pool(name="const", bufs=1) as cp, \
         tc.tile_pool(name="in", bufs=4) as inp, \
         tc.tile_pool(name="io", bufs=3) as iop, \
         tc.tile_pool(name="mid", bufs=3) as mp, \
         tc.tile_pool(name="pm", bufs=2, space="PSUM") as pmp, \
         tc.tile_pool(name="pe", bufs=2, space="PSUM") as pep:
        lt = cp.tile([128, 128], BF16)
        nc.sync.dma_start(out=lt[:], in_=lt_d)
        for g in range(N_GROUPS):
            sl = slice(g * G, (g + 1) * G)
            ct = inp.tile([128, G, 128], BF16, tag="in")
            nc.sync.dma_start(out=ct[:], in_=x_d[:, sl, :])
            cd_in = list(ct[:].ap[1])           # [128, G] image pitch

            # pairsum in PSUM: P_main = LQ @ x_even (+)= LQ @ x_odd,
            # 512-free matmuls (one PSUM bank each), strided bf16 rhs
            t1 = pmp.tile([128, G, 64], F32, tag="t1")
            for h in range(2):
                cs = slice(8 * h, 8 * (h + 1))
                nc.tensor.matmul(t1[:, cs, :], lt[:], ct[:, cs, 0:128:2],
                                 start=True, stop=False)
                nc.tensor.matmul(t1[:, cs, :], lt[:], ct[:, cs, 1:128:2],
                                 start=False, stop=True)
            # edge columns: E[c,{0,1}] = LQ @ x[:,{0,127}]
            te = pep.tile([128, G, 2], F32, tag="te")
            nc.tensor.matmul(
                te[:], lt[:],
                _as_strided(ct[:, :, 0:1],
                            [list(ct[:].ap[0]), cd_in, [127, 2]]),
                start=True, stop=True)

            # evacuate P to SBUF bf16: psb cols 2..65 main (ACT half /
            # DVE half), cols {0,1,66,67} = 2x edge (ACT broadcast)
            pt = mp.tile([128, G, 68], BF16, tag="P")
            gdim = list(pt[:].ap[1])            # [68, G]
            pdim0 = list(pt[:].ap[0])           # partition dim
            nc.scalar.copy(out=pt[:, 0:8, 2:66], in_=t1[:, 0:8, :])
            nc.vector.tensor_copy(out=pt[:, 8:16, 2:66], in_=t1[:, 8:16, :])
            ed = te[:].ap
            nc.scalar.mul(
                _as_strided(pt[:, :, 0:1], [pdim0, gdim, [66, 2], [1, 2]]),
                _as_strided(te[:], [list(ed[0]), list(ed[1]), [1, 2], [0, 2]]),
                2.0)

            # fold: out[4a+b] = P[2a+b] + P[2a+b+2], overlapping
            # as-strided reads, contiguous bf16 write of cols 4..123,
            # split DVE (a=0..15) / GpSimd (a=16..29). Edge cols
            # {0..3,124..127} = 2x P{0..3,64..67} on ACT.
            ot = iop.tile([128, G, 128], BF16, tag="out")
            odim = ot[:].ap
            na = DVE_A
            in0a = _as_strided(pt[:, :, 2:3], [pdim0, gdim, [2, na], [1, 4]])
            in1a = _as_strided(pt[:, :, 4:5], [pdim0, gdim, [2, na], [1, 4]])
            outa = _as_strided(
                ot[:, :, 4:5], [list(odim[0]), list(odim[1]), [4, na], [1, 4]])
            nc.vector.tensor_add(out=outa, in0=in0a, in1=in1a)
            nb = 30 - na
            in0b = _as_strided(
                pt[:, :, 2 + 2 * na:3 + 2 * na],
                [pdim0, gdim, [2, nb], [1, 4]])
            in1b = _as_strided(
                pt[:, :, 4 + 2 * na:5 + 2 * na],
                [pdim0, gdim, [2, nb], [1, 4]])
            outb = _as_strided(
                ot[:, :, 4 + 4 * na:5 + 4 * na],
                [list(odim[0]), list(odim[1]), [4, nb], [1, 4]])
            nc.gpsimd.tensor_add(out=outb, in0=in0b, in1=in1b)
            edge_in = _as_strided(pt[:, :, 0:1], [pdim0, gdim, [64, 2], [1, 4]])
            edge_out = _as_strided(
                ot[:, :, 0:1], [list(odim[0]), list(odim[1]), [124, 2], [1, 4]])
            nc.scalar.mul(edge_out, edge_in, 2.0)

            # stores ride the ACT HWDGE ring so they never FIFO behind
            # upcoming loads on the SP ring
            nc.scalar.dma_start(out=o_d[:, sl, :], in_=ot[:])


_CACHE = {}


def _get_nc():
    if "nc" not in _CACHE:
        nc = bacc.Bacc("TRN2", target_bir_lowering=False, debug=False)
        x_d = nc.dram_tensor("x", (128, C_PER_CORE, 128), BF16,
                             kind="ExternalInput").ap()
        lt_d = nc.dram_tensor("lt", (128, 128), BF16,
                              kind="ExternalInput").ap()
        o_d = nc.dram_tensor("o", (128, C_PER_CORE, 128), BF16,
                             kind="ExternalOutput").ap()
        with tile.TileContext(nc) as tc:
            _dpconv_tile(tc, o_d, x_d, lt_d)
        nc.compile()
        _CACHE["nc"] = nc
    return _CACHE["nc"]


def _stage(xk: np.ndarray) -> np.ndarray:
    """[C,H,W] f32 -> [H,C,W] bf16 (H-major so DMA lines are contiguous)."""
    return np.ascontiguousarray(xk.transpose(1, 0, 2)).astype(BF16_NP)


def run(x: np.ndarray, **spmd_kwargs) -> bass_utils.BassKernelResults:
    """Shard x (8,64,128,128) across 8 cores and run the Bass kernel."""
    nc = _get_nc()
    in_maps = [
        {"x": _stage(x[k]), "lt": _LQ_T_BF16} for k in range(N_CORES)
    ]
    return bass_utils.run_bass_kernel_spmd(
        nc, in_maps, core_ids=list(range(N_CORES)), **spmd_kwargs)


def kernel(x) -> np.ndarray:
    x = np.asarray(x, dtype=np.float32)
    assert x.shape == (N_CORES, C_PER_CORE, 128, 128), x.shape
    res = run(x)
    return np.stack(
        [res.results[k]["o"].astype(np.float32).transpose(1, 0, 2)
         for k in range(N_CORES)],
        axis=0)
